# revision 1
# baseline (speedup 1.0000x reference)
"""Trainium2 Bass kernel for the layered-circuit WMC problem.

Computation (see reference): vals = [weights | neg_weights]  # [B, 8192]
12 alternating AND(prod)/OR(sum) layers, each gathering 2 children per node
from the previous layer's 8192 node values, then a final sum over nodes.

Sharding: data-parallel over batch. 8 cores x 128 batch rows each.
Layout: batch on SBUF partitions [128, 8192]; the per-layer gather runs on
GPSIMD (ap_gather, fp32 SBUF free-axis gather with indices shared across
partitions); the 2-child combine runs on the Vector engine; final node-sum
via vector tensor_reduce.

The index tensors are runtime data (DMAed in), so the compiled NEFF is
input-independent and cached across calls.
"""

import numpy as np

N_LAYERS = 12
WIDTH = 8192
N_VARS = 4096
BATCH = 1024
N_CORES = 8
PB = BATCH // N_CORES  # 128 batch rows per core
NUM_IDXS = 2 * WIDTH  # both children gathered in one call
IDXF = NUM_IDXS // 16  # int16 elements per partition per layer

_CACHE = {}


def _build_nc():
    import concourse.bacc as bacc
    import concourse.mybir as mybir
    from concourse import library_config

    f32 = mybir.dt.float32
    i16 = mybir.dt.int16

    nc = bacc.Bacc("TRN2", target_bir_lowering=False, debug=False)

    leaves = nc.dram_tensor("leaves", [PB, WIDTH], f32, kind="ExternalInput")
    idxs = nc.dram_tensor("idxs", [PB, N_LAYERS * IDXF], i16, kind="ExternalInput")
    out = nc.dram_tensor("out", [PB, 1], f32, kind="ExternalOutput")

    with (
        nc.sbuf_tensor("v0", [PB, WIDTH], f32) as v0,
        nc.sbuf_tensor("v1", [PB, WIDTH], f32) as v1,
        nc.sbuf_tensor("gath", [PB, NUM_IDXS], f32) as gath,
        nc.sbuf_tensor("idx_sb", [PB, N_LAYERS * IDXF], i16) as idx_sb,
        nc.sbuf_tensor("res", [PB, 1], f32) as res,
        nc.semaphore("io") as io,
        nc.semaphore("gsem") as gsem,
        nc.semaphore("vsem") as vsem,
        nc.Block() as block,
    ):
        vals = [v0, v1]

        @block.gpsimd
        def _(g):
            g.load_library(library_config.ap_gather)
            g.dma_start(v0[:], leaves[:]).then_inc(io, 16)
            g.dma_start(idx_sb[:], idxs[:]).then_inc(io, 16)
            g.wait_ge(io, 32)
            for l in range(N_LAYERS):
                if l > 0:
                    # combine l-1 done: vals[l%2] ready, gath free to overwrite
                    g.wait_ge(vsem, l)
                g.ap_gather(
                    out_ap=gath[:].rearrange("p (n d) -> p n d", d=1),
                    in_ap=vals[l % 2][:].rearrange("p (n d) -> p n d", d=1),
                    idxs_ap=idx_sb[:, l * IDXF : (l + 1) * IDXF],
                    channels=128,
                    num_elems=WIDTH,
                    d=1,
                    num_idxs=NUM_IDXS,
                ).then_inc(gsem, 1)

        @block.vector
        def _(v):
            for l in range(N_LAYERS):
                v.wait_ge(gsem, l + 1)
                op = mybir.AluOpType.mult if l % 2 == 0 else mybir.AluOpType.add
                v.tensor_tensor(
                    out=vals[(l + 1) % 2][:],
                    in0=gath[:, :WIDTH],
                    in1=gath[:, WIDTH:],
                    op=op,
                ).then_inc(vsem, 1)
            v.tensor_reduce(
                out=res[:],
                in_=vals[N_LAYERS % 2][:],
                axis=mybir.AxisListType.X,
                op=mybir.AluOpType.add,
            ).then_inc(vsem, 1)

        @block.sync
        def _(s):
            s.wait_ge(vsem, N_LAYERS + 1)
            s.dma_start(out[:], res[:]).then_inc(io, 16)
            s.wait_ge(io, 48)

    nc.compile()
    return nc


def _get_nc():
    if "nc" not in _CACHE:
        _CACHE["nc"] = _build_nc()
    return _CACHE["nc"]


def _prep_inputs(weights, neg_weights, children):
    leaves_full = np.concatenate(
        [np.asarray(weights, np.float32), np.asarray(neg_weights, np.float32)], axis=1
    )  # [1024, 8192]
    ch = np.asarray(children)
    idx_blocks = []
    for l in range(N_LAYERS):
        idx_list = np.concatenate([ch[l, :, 0], ch[l, :, 1]]).astype(np.int16)
        # wrapped layout: index j -> partition j%16 (replicated over 8 cores),
        # int16 free position j//16
        wrapped = np.tile(idx_list.reshape(IDXF, 16).T, (N_CORES, 1))  # [128, IDXF]
        idx_blocks.append(wrapped)
    idx_arr = np.ascontiguousarray(np.concatenate(idx_blocks, axis=1))  # [128, 12*IDXF]
    in_maps = [
        {
            "leaves": np.ascontiguousarray(leaves_full[c * PB : (c + 1) * PB]),
            "idxs": idx_arr,
        }
        for c in range(N_CORES)
    ]
    return in_maps


def run(weights, neg_weights, children, trace=False):
    from concourse.bass_utils import run_bass_kernel_spmd

    nc = _get_nc()
    in_maps = _prep_inputs(weights, neg_weights, children)
    br = run_bass_kernel_spmd(
        nc, in_maps, list(range(N_CORES)), trace=trace
    )
    out = np.concatenate([r["out"][:, 0] for r in br.results]).astype(np.float32)
    return out, br


def kernel(weights, neg_weights, children):
    out, _ = run(weights, neg_weights, children)
    return out
